# revision 56
# baseline (speedup 1.0000x reference)
"""Dense GAT (2-layer, 8+1 heads) on 8 Trainium2 NeuronCores — V3.

Row-parallel over destination rows. Key algebraic trick vs V2: softmax over j
is invariant to any per-destination factor, so divide e by exp(ad_i)·exp(as_j):
  e[j,i] = adjT * max(u_j U_i, p_j P_i)           (u=exp(as), U=exp(ad), ...)
  ẽ[j,i] = e/(u_j U_i) = adjT * max(1, w_j Q_i)   w=exp(-.8 as), Q=exp(-.8 ad)
The U_i factor cancels in the softmax; the u_j factor rides in the matmul lhsT
(u-scaled h1 / u-column) or in the GPSIMD mask op. Per-tile chains:
  A+D: mq = ts(max(w_j*Q_bc, 1))      [DVE 4x]  ; e = mq*adjT   [DVE]
       -> matmul vs [u*h1 | u]
  A+G: same mq                        [DVE 4x]  ; e = mq*adjT [GPSIMD tt]
       -> matmul vs [u*h1 | u]
  B:   t1 = Relu(-0.8 adb - 0.8 as_j) [ACT]     ; t2 = Exp(t1 + as_j) = u_j*ẽ'
       ; e = t2*adjT [DVE or GPSIMD]  -> matmul vs [h1 | 1]
All three elementwise engines (DVE, ACT, GPSIMD) run balanced. adj arrives
pre-transposed fp16 from the host (no PE transposes). One fp16 AllGather of
[512, 130] (h2-fp16 | as2 | ad2) between layers.
"""
import numpy as np

N = 4096
F_IN = 256
HID = 64
H1 = 8
F1 = H1 * HID
OUT = 128
N_CORES = 8
R = N // N_CORES
JT = N // 128
IT = R // 128
NEG_ATT = 0.2
NEG_OUT = 0.01
G2 = OUT + 2  # gather payload words per row: h2(128) | as2 | ad2

# --- engine-assignment knobs (tuned against TimelineSim) ---
ACT_16THS = 4      # fraction of tiles using the ACT e-chain (B)
GPS_16THS = 5      # fraction of masks on GPSIMD
H1COPY = "AAD"     # engine cycle for h1 psum->sbuf copies (GPSIMD cannot
                   # access PSUM, so only ACT/DVE here)
L2_ACT_16THS = 6
L2_GPS_16THS = 5
KEEPWARM = 0     # chained dummy matmuls spanning the collective, keeping
                   # the PE's HAM clock-gate at full rate for the L2 phase

_CACHE = {}


def _is_act(h, jt):
    return ((h * JT + jt) * 5 + 2) % 16 < ACT_16THS


GPS_EXCL = ()      # jt values excluded from GPSIMD masks


def _is_gps_mask(h, jt):
    if jt in GPS_EXCL:
        return False
    return ((h * JT + jt) * 11 + 5) % 16 < GPS_16THS


def _l2_is_act(jt):
    # first tiles stay on the fast DVE chain so the cold PE gets a dense
    # run of matmuls to ramp on
    if jt < 6:
        return False
    return (jt * 5 + 2) % 16 < L2_ACT_16THS


def _l2_is_gps(jt):
    if jt < 6:
        return False
    return (jt * 11 + 5) % 16 < L2_GPS_16THS


def _build():
    import concourse.bass as bass
    from concourse import bacc
    import concourse.mybir as mybir
    import concourse.tile as tile

    f32 = mybir.dt.float32
    f16 = mybir.dt.float16
    A = mybir.ActivationFunctionType
    Al = mybir.AluOpType

    nc = bacc.Bacc("TRN2", target_bir_lowering=False, debug=False,
                   num_devices=N_CORES)
    # packed weights blob: [w1(2x512) | vsd1(2x16) | rhs2(4x130)] per 128-row
    WBLOB = 2 * F1 + 2 * (2 * H1) + 4 * G2
    d_x16T = nc.dram_tensor("x16T", [F_IN, N], f16, kind="ExternalInput")
    d_xm16T = nc.dram_tensor("xm16T", [F_IN, R], f16, kind="ExternalInput")
    d_adjT = nc.dram_tensor("adjT", [N, R], f16, kind="ExternalInput")
    d_wblob = nc.dram_tensor("wblob", [128, WBLOB], f16, kind="ExternalInput")
    d_b1c = nc.dram_tensor("b1c", [HID, H1], f32, kind="ExternalInput")
    d_b2c = nc.dram_tensor("b2c", [OUT, 1], f32, kind="ExternalInput")
    d_outT = nc.dram_tensor("outT", [OUT, R], f32, kind="ExternalOutput")

    with tile.TileContext(nc) as tc:
        with tc.tile_pool(name="const", bufs=1) as const, \
             tc.tile_pool(name="big", bufs=1) as big, \
             tc.tile_pool(name="work", bufs=3) as work, \
             tc.tile_pool(name="dram", bufs=1, space="DRAM") as dram, \
             tc.tile_pool(name="ps_h", bufs=2, space="PSUM") as ps_h, \
             tc.tile_pool(name="ps_sm", bufs=2, space="PSUM") as ps_sm, \
             tc.tile_pool(name="ps_small", bufs=2, space="PSUM") as ps_small, \
             tc.tile_pool(name="ps_ag", bufs=2, space="PSUM") as ps_ag:
            ones_row = const.tile([1, 128], f32)
            nc.vector.memset(ones_row, 1.0)
            ones_row16 = const.tile([1, 128], f16)
            nc.vector.memset(ones_row16, 1.0)
            ones_col16 = const.tile([128, 1], f16)
            nc.vector.memset(ones_col16, 1.0)
            wblob_sb = const.tile([128, WBLOB], f16)
            nc.sync.dma_start(out=wblob_sb, in_=d_wblob[:, :])
            w1_sb = wblob_sb[:, 0:2 * F1].rearrange("p (k f) -> p k f", k=2)
            vsd1_sb = wblob_sb[:, 2 * F1:2 * F1 + 4 * H1].rearrange(
                "p (k f) -> p k f", k=2)
            rhs2_sb = wblob_sb[:, 2 * F1 + 4 * H1:WBLOB].rearrange(
                "p (k f) -> p k f", k=4)
            b1_sb = const.tile([HID, H1], f32)
            nc.sync.dma_start(out=b1_sb, in_=d_b1c[:, :])
            b2_sb = const.tile([OUT, 1], f32)
            nc.sync.dma_start(out=b2_sb, in_=d_b2c[:, :])

            adjT_all = big.tile([128, JT, R], f16)          # 32 KB/part
            h1_all = big.tile([128, JT, H1, HID + 1], f16)  # 33.3 KB/part
            h1u_all = big.tile([128, JT, H1, HID + 1], f16)
            asrc_all = big.tile([128, JT, H1], f32)
            nas_all = big.tile([128, JT, H1], f32)   # -0.8*as
            u_all = big.tile([128, JT, H1], f32)     # exp(as)
            w_all = big.tile([128, JT, H1], f32)     # exp(-0.8*as)
            x2T_all = big.tile([128, 4, R], f16)
            adstT = big.tile([H1, R], f32)
            adst_rows = big.tile([1, H1, R], f32)
            adst2T = big.tile([1, R], f32)

            nc.vector.memset(h1_all[:, :, :, HID:HID + 1], 1.0)

            # ---- a_dst for own rows: ps_adT = v_dst1^T @ xm directly ----
            xm16 = big.tile([128, 2, R], f16)
            nc.sync.dma_start(out=xm16[:, 0, :], in_=d_xm16T[0:128, :])
            nc.sync.dma_start(out=xm16[:, 1, :], in_=d_xm16T[128:256, :])
            ps_adT = ps_ag.tile([H1, R], f32, tag="agg")
            for kb in range(2):
                nc.tensor.matmul(ps_adT, vsd1_sb[:, kb, H1:2 * H1],
                                 xm16[:, kb, :],
                                 start=(kb == 0), stop=(kb == 1))
            nc.vector.tensor_copy(adstT, ps_adT)
            # rows for heads 0-1 early (they gate the first broadcasts);
            # rows 2-7 are emitted after the bulk loads to keep the HWDGE
            # descriptor FIFO flowing
            for h in range(2):
                nc.sync.dma_start(out=adst_rows[:, h, :],
                                  in_=adstT[h:h + 1, :])

            def _pre_head(h):
                ps_b = ps_sm.tile([128, R], f32, tag="sm", name=f"ps_b{h}")
                nc.tensor.matmul(ps_b, ones_row, adst_rows[:, h, :],
                                 start=True, stop=True)
                adb_h = work.tile([128, R], f32, tag="adb", bufs=2,
                                  name=f"adb{h}")
                nc.scalar.copy(adb_h, ps_b)
                qrow = work.tile([1, R], f16, tag="qrow", bufs=2,
                                 name=f"qrow{h}")
                nc.scalar.activation(qrow, adst_rows[:, h, :], A.Exp,
                                     scale=-0.8)
                ps_q = ps_sm.tile([128, R], f32, tag="sm", name=f"ps_q{h}")
                nc.tensor.matmul(ps_q, ones_row16, qrow, start=True, stop=True)
                qbc_h = work.tile([128, R], f16, tag="qbc", bufs=2,
                                  name=f"qbc{h}")
                nc.scalar.copy(qbc_h, ps_q)
                return adb_h, qbc_h

            pre = _pre_head(0)
            # ---- h1 | a_src production ----
            # bulk loads, emission-ordered for streaming: adjT slab 0 first
            # (head 0 needs it soonest), then x16 quarters interleaved with
            # the remaining adjT slabs (h1[jt] needs both kb-halves).
            x16_sb = big.tile([128, 2, N], f16)        # 16 KB/part

            def _adj_slab(sl):
                nc.sync.dma_start(
                    out=adjT_all[:, sl * 8:(sl + 1) * 8, :],
                    in_=d_adjT[sl * 8 * 128:(sl + 1) * 8 * 128, :].rearrange(
                        "(t p) r -> p t r", p=128))

            _adj_slab(0)
            for q in range(4):
                qs = slice(q * (N // 4), (q + 1) * (N // 4))
                nc.sync.dma_start(out=x16_sb[:, 0, qs], in_=d_x16T[0:128, qs])
                nc.sync.dma_start(out=x16_sb[:, 1, qs],
                                  in_=d_x16T[128:256, qs])
                if q < 3:
                    _adj_slab(q + 1)
            for h in range(2, H1):
                nc.sync.dma_start(out=adst_rows[:, h, :],
                                  in_=adstT[h:h + 1, :])
            h1cpy = 0
            for jt in range(JT):
                cols = slice(jt * 128, (jt + 1) * 128)
                ps_h1 = ps_h.tile([128, F1], f32, tag="h")
                ps_a = ps_small.tile([128, H1], f32, tag="sm")
                for kb in range(2):
                    nc.tensor.matmul(ps_h1, x16_sb[:, kb, cols],
                                     w1_sb[:, kb, :],
                                     start=(kb == 0), stop=(kb == 1))
                    nc.tensor.matmul(ps_a, x16_sb[:, kb, cols],
                                     vsd1_sb[:, kb, 0:H1],
                                     start=(kb == 0), stop=(kb == 1))
                eng = H1COPY[h1cpy % len(H1COPY)]
                h1cpy += 1
                dst = h1_all[:, jt, :, 0:HID]
                src = ps_h1.rearrange("p (h c) -> p h c", c=HID)
                if eng == "A":
                    nc.scalar.copy(dst, src)
                elif eng == "G":
                    nc.gpsimd.tensor_copy(dst, src)
                else:
                    nc.vector.tensor_copy(dst, src)
                nc.vector.tensor_copy(asrc_all[:, jt, :], ps_a)
                if jt % 8 == 7:
                    g = slice(jt - 7, jt + 1)
                    nc.scalar.activation(u_all[:, g, :], asrc_all[:, g, :],
                                         A.Exp)
                    nc.scalar.activation(w_all[:, g, :], asrc_all[:, g, :],
                                         A.Exp, scale=-0.8)
                    nc.scalar.activation(nas_all[:, g, :], asrc_all[:, g, :],
                                         A.Copy, scale=-0.8)

            # ---- layer-1 attention ----
            for h in range(H1):
                adb_h, qbc_h = pre
                if h + 1 < H1:
                    pre = _pre_head(h + 1)
                ps_agg = ps_ag.tile([HID + 1, R], f32, tag="agg")
                for jt in range(JT):
                    e = work.tile([128, R], f16, tag="e", bufs=6)
                    gps = _is_gps_mask(h, jt)
                    if _is_act(h, jt):
                        t1 = work.tile([128, R], f32, tag="t1", bufs=2)
                        nc.scalar.activation(t1, adb_h, A.Relu,
                                             bias=nas_all[:, jt, h:h + 1],
                                             scale=-0.8)
                        t2 = work.tile([128, R], f16, tag="t2", bufs=3)
                        nc.scalar.activation(t2, t1, A.Exp,
                                             bias=asrc_all[:, jt, h:h + 1])
                        if gps:
                            nc.gpsimd.tensor_mul(e, t2, adjT_all[:, jt, :])
                        else:
                            nc.vector.tensor_mul(e, t2, adjT_all[:, jt, :])
                        lhsT = h1_all[:, jt, h, :]
                    else:
                        # just-in-time u-scaled lhsT (emitting these in the
                        # h1 loop would queue ~17us of DVE work ahead of the
                        # first attention tiles)
                        nc.vector.tensor_scalar_mul(h1u_all[:, jt, h, :],
                                                    h1_all[:, jt, h, :],
                                                    u_all[:, jt, h:h + 1])
                        mq = work.tile([128, R], f16, tag="mq", bufs=5)
                        nc.vector.tensor_scalar(mq, qbc_h,
                                                w_all[:, jt, h:h + 1], 1.0,
                                                op0=Al.mult, op1=Al.max)
                        if gps:
                            nc.gpsimd.tensor_mul(e, mq, adjT_all[:, jt, :])
                        else:
                            nc.vector.tensor_mul(e, mq, adjT_all[:, jt, :])
                        lhsT = h1u_all[:, jt, h, :]
                    nc.tensor.matmul(ps_agg, lhsT, e,
                                     start=(jt == 0), stop=(jt == JT - 1))
                rz = work.tile([1, R], f32, tag="rz", bufs=2)
                nc.vector.reciprocal(rz, ps_agg[HID:HID + 1, :])
                ps_rzb = ps_sm.tile([HID, R], f32, tag="sm")
                nc.tensor.matmul(ps_rzb, ones_row[:, 0:HID], rz,
                                 start=True, stop=True)
                rzb = work.tile([HID, R], f32, tag="rzbs", bufs=2)
                nc.scalar.copy(rzb, ps_rzb)
                y_h = work.tile([HID, R], f32, tag="yh", bufs=1)
                nc.vector.tensor_mul(y_h, ps_agg[0:HID, :], rzb)
                po = (h % 2) * HID
                nc.scalar.activation(
                    x2T_all[po:po + HID, h // 2, :], y_h, A.Prelu,
                    bias=b1_sb[:, h:h + 1], alpha=NEG_OUT)

            # ---- layer 2 ----
            bounce_in = dram.tile([R, G2], f16)
            bounce_out = dram.tile([N_CORES, R, G2], f16, addr_space="Shared")
            ps_adT2 = ps_small.tile([1, R], f32, tag="sm")
            for kt in range(4):
                nc.tensor.matmul(ps_adT2, rhs2_sb[:, kt, OUT + 1:OUT + 2],
                                 x2T_all[:, kt, :],
                                 start=(kt == 0), stop=(kt == 3))
            nc.vector.tensor_copy(adst2T, ps_adT2)
            for it in range(IT):
                ps_h2 = ps_h.tile([128, G2], f32, tag="h")
                for kt in range(4):
                    nc.tensor.matmul(
                        ps_h2, x2T_all[:, kt, it * 128:(it + 1) * 128],
                        rhs2_sb[:, kt, :],
                        start=(kt == 0), stop=(kt == 3))
                h2m = work.tile([128, G2], f16, tag="h2m", bufs=2)
                nc.vector.tensor_copy(h2m, ps_h2)
                nc.sync.dma_start(
                    out=bounce_in[it * 128:(it + 1) * 128, :], in_=h2m)
            nc.gpsimd.collective_compute(
                "AllGather",
                bass.mybir.AluOpType.bypass,
                replica_groups=[list(range(N_CORES))],
                ins=[bounce_in.opt()],
                outs=[bounce_out.opt()],
            )
            # Keep the PE's HAM clock-gate warm through the collective: a
            # serial accumulate chain of junk matmuls (gated on the last h2m
            # tile so it starts at gather time, not earlier).
            if KEEPWARM:
                ps_keep = ps_small.tile([1, R], f32, tag="sm")
                for k in range(KEEPWARM):
                    nc.tensor.matmul(ps_keep, ones_row[:, 0:1], adst2T,
                                     start=(k == 0), stop=(k == KEEPWARM - 1))
            h2g_all = big.tile([128, JT, G2], f16)   # 8.3 KB/part
            for cc in range(4):
                nc.sync.dma_start(
                    out=h2g_all[:, cc * 8:(cc + 1) * 8, :],
                    in_=bounce_out[2 * cc:2 * cc + 2].rearrange(
                        "c (t p) g -> p (c t) g", p=128))

            # own-row broadcasts for L2
            adb2 = work.tile([128, R], f32, tag="adb", bufs=2)
            ps_b2 = ps_sm.tile([128, R], f32, tag="sm")
            nc.tensor.matmul(ps_b2, ones_row, adst2T, start=True, stop=True)
            nc.scalar.copy(adb2, ps_b2)
            q2row = work.tile([1, R], f16, tag="qrow", bufs=2)
            nc.scalar.activation(q2row, adst2T, A.Exp, scale=-0.8)
            ps_q2 = ps_sm.tile([128, R], f32, tag="sm")
            nc.tensor.matmul(ps_q2, ones_row16, q2row, start=True, stop=True)
            q2bc = work.tile([128, R], f16, tag="qbc", bufs=2)
            nc.scalar.copy(q2bc, ps_q2)

            as2f = big.tile([128, JT], f32)
            nas2 = big.tile([128, JT], f32)
            u2_all = big.tile([128, JT], f32)
            w2_all = big.tile([128, JT], f32)
            u2c16 = big.tile([128, JT], f16)
            h2u_all = big.tile([128, JT, OUT + 1], f16)
            for cc in range(4):
                g = slice(cc * 8, (cc + 1) * 8)
                asg = h2g_all[:, g, OUT:OUT + 1].rearrange(
                    "p t one -> p (t one)")
                nc.vector.tensor_copy(as2f[:, g], asg)
                nc.scalar.activation(nas2[:, g], as2f[:, g], A.Copy,
                                     scale=-0.8)
                nc.scalar.activation(u2_all[:, g], as2f[:, g], A.Exp)
                nc.scalar.activation(w2_all[:, g], as2f[:, g], A.Exp,
                                     scale=-0.8)
                nc.vector.tensor_copy(u2c16[:, g], u2_all[:, g])
                nc.vector.memset(h2g_all[:, g, OUT:OUT + 1], 1.0)
                for jt in range(cc * 8, (cc + 1) * 8):
                    if not _l2_is_act(jt):
                        nc.vector.tensor_scalar_mul(h2u_all[:, jt, :],
                                                    h2g_all[:, jt, 0:OUT + 1],
                                                    u2_all[:, jt:jt + 1])

            ps_o2a = ps_ag.tile([128, R], f32, tag="agg")
            ps_o2b = ps_ag.tile([128, R], f32, tag="agg")
            ps_z2a = ps_small.tile([1, R], f32, tag="sm")
            ps_z2b = ps_small.tile([1, R], f32, tag="sm")
            for jt in range(JT):
                ps_o2 = ps_o2a if jt % 2 == 0 else ps_o2b
                ps_z2 = ps_z2a if jt % 2 == 0 else ps_z2b
                e2 = work.tile([128, R], f16, tag="e", bufs=6)
                gps = _l2_is_gps(jt)
                if _l2_is_act(jt):
                    t1 = work.tile([128, R], f32, tag="t1", bufs=2)
                    nc.scalar.activation(t1, adb2, A.Relu,
                                         bias=nas2[:, jt:jt + 1], scale=-0.8)
                    t2 = work.tile([128, R], f16, tag="t2", bufs=3)
                    nc.scalar.activation(t2, t1, A.Exp,
                                         bias=as2f[:, jt:jt + 1])
                    if gps:
                        nc.gpsimd.tensor_mul(e2, t2, adjT_all[:, jt, :])
                    else:
                        nc.vector.tensor_mul(e2, t2, adjT_all[:, jt, :])
                    lhsT, zcol = h2g_all[:, jt, 0:OUT], ones_col16
                else:
                    mq = work.tile([128, R], f16, tag="mq", bufs=5)
                    nc.vector.tensor_scalar(mq, q2bc, w2_all[:, jt:jt + 1],
                                            1.0, op0=Al.mult, op1=Al.max)
                    if gps:
                        nc.gpsimd.tensor_mul(e2, mq, adjT_all[:, jt, :])
                    else:
                        nc.vector.tensor_mul(e2, mq, adjT_all[:, jt, :])
                    lhsT, zcol = h2u_all[:, jt, 0:OUT], u2c16[:, jt:jt + 1]
                nc.tensor.matmul(ps_o2, lhsT, e2,
                                 start=(jt < 2), stop=(jt >= JT - 2))
                nc.tensor.matmul(ps_z2, zcol, e2,
                                 start=(jt < 2), stop=(jt >= JT - 2))
            z2b_sb = work.tile([1, R], f32, tag="rz", bufs=2)
            nc.vector.tensor_copy(z2b_sb, ps_z2b[0:1, :])
            nc.vector.tensor_add(z2b_sb, ps_z2a[0:1, :], z2b_sb)
            rz2 = work.tile([1, R], f32, tag="rz", bufs=2)
            nc.vector.reciprocal(rz2, z2b_sb)
            ps_rz2b = ps_sm.tile([128, R], f32, tag="sm")
            nc.tensor.matmul(ps_rz2b, ones_row, rz2, start=True, stop=True)
            rz2b = work.tile([128, R], f32, tag="rz2bs", bufs=1)
            nc.scalar.copy(rz2b, ps_rz2b)
            o2b_sb = work.tile([128, R], f32, tag="o2s", bufs=1)
            nc.scalar.copy(o2b_sb, ps_o2b)
            o2 = work.tile([128, R], f32, tag="adb", bufs=2)
            nc.vector.tensor_add(o2, ps_o2a, o2b_sb)
            nc.vector.tensor_mul(o2, o2, rz2b)
            outT_sb = work.tile([OUT, R], f32, tag="outT", bufs=1)
            nc.scalar.activation(outT_sb, o2, A.Prelu,
                                 bias=b2_sb[:, 0:1], alpha=NEG_OUT)
            nc.sync.dma_start(out=d_outT[:, :], in_=outT_sb)

    nc.finalize()
    return nc


def _prep_host(x, adj, w1, att_src1, att_dst1, b1, w2, att_src2, att_dst2, b2):
    x = np.asarray(x, np.float32).reshape(N, F_IN)
    adj = np.asarray(adj, np.float32).reshape(N, N)
    w1 = np.asarray(w1, np.float32)
    w2 = np.asarray(w2, np.float32)
    att_src1 = np.asarray(att_src1, np.float32)
    att_dst1 = np.asarray(att_dst1, np.float32)
    att_src2 = np.asarray(att_src2, np.float32)
    att_dst2 = np.asarray(att_dst2, np.float32)
    b1 = np.asarray(b1, np.float32)
    b2 = np.asarray(b2, np.float32)

    x16T = np.ascontiguousarray(x.T.astype(np.float16))
    adjT16 = np.ascontiguousarray(adj.T.astype(np.float16))
    v_src1 = np.empty((F_IN, H1), np.float32)
    v_dst1 = np.empty((F_IN, H1), np.float32)
    for h in range(H1):
        blk = w1[:, h * HID:(h + 1) * HID]
        v_src1[:, h] = blk @ att_src1[h]
        v_dst1[:, h] = blk @ att_dst1[h]
    v_src2 = (w2 @ att_src2[0])[:, None]
    v_dst2 = (w2 @ att_dst2[0])[:, None]
    rhs2 = np.concatenate([w2, v_src2, v_dst2], axis=1)  # [512, 130]
    # pack all fp16 weights into one [128, WBLOB] blob matching the SBUF view
    w1_kb = w1.reshape(2, 128, F1).transpose(1, 0, 2)            # [128,2,512]
    vsd1_kb = np.concatenate([v_src1, v_dst1], axis=1).reshape(
        2, 128, 2 * H1).transpose(1, 0, 2)                        # [128,2,16]
    rhs2_kb = rhs2.reshape(4, 128, G2).transpose(1, 0, 2)        # [128,4,130]
    wblob = np.concatenate([
        w1_kb.reshape(128, -1), vsd1_kb.reshape(128, -1),
        rhs2_kb.reshape(128, -1)], axis=1).astype(np.float16)
    wblob = np.ascontiguousarray(wblob)
    b1c = np.ascontiguousarray(b1.reshape(H1, HID).T)
    b2c = np.ascontiguousarray(b2.reshape(OUT, 1))

    in_maps = []
    for c in range(N_CORES):
        rows = slice(c * R, (c + 1) * R)
        in_maps.append({
            "x16T": x16T,
            "xm16T": np.ascontiguousarray(x16T[:, rows]),
            "adjT": np.ascontiguousarray(adjT16[:, rows]),
            "wblob": wblob,
            "b1c": b1c,
            "b2c": b2c,
        })
    return in_maps


def kernel(**inputs) -> np.ndarray:
    from concourse.bass_utils import run_bass_kernel_spmd

    if "nc" not in _CACHE:
        _CACHE["nc"] = _build()
    nc = _CACHE["nc"]
    in_maps = _prep_host(**inputs)
    try:
        res = run_bass_kernel_spmd(nc, in_maps, list(range(N_CORES)))
    except Exception:
        # transient NRT device wedge — one clean retry
        res = run_bass_kernel_spmd(nc, in_maps, list(range(N_CORES)))
    out = np.empty((1, N, OUT), np.float32)
    for c in range(N_CORES):
        out[0, c * R:(c + 1) * R, :] = res.results[c]["outT"].T
    return out
